# revision 61
# baseline (speedup 1.0000x reference)
"""Causal self-attention (B=2, T=2048, E=1024, H=16) on 8 trn2 NeuronCores.

Sharding: core = b*4 + g  (b = batch index, g = head-group of 4 heads).
Each core computes its 4 heads' attention for its batch plus a partial
output projection; the host sums the 4 partials per batch.

v3 structure (single software-pipelined loop):
  All inputs arrive as a few ~0.5-1MB contiguous DMAs on the sync HWDGE
  ring, ordered by first use (host pre-lays x^T window-major / kk-major
  so every transfer is one dense block); outputs also use the sync ring,
  which is idle by the time the first result is ready (keeping them off
  the scalar queue protects the exp cadence from DMA-semaphore waits).
  The prologue computes only window-0 projections (v + q/k with RoPE via
  a block-diag permutation matmul).  The attention loop (windows j=0..3,
  k-chunks c) is paced by the ACT exp cadence; everything else rides in
  its slack as side tasks with deadlines: q/k projections for window
  j+1, window j's own v projection, output projections for window j-1,
  and the trailing AV matmuls + softmax-normalize of window j-1 (moved
  into window j's first chunks so next-window scores never wait on
  them).  Scores are computed transposed (S^T = K Q^T) with two heads
  row-packed per matmul; each head's V tile carries 64 appended ones
  columns so the AV matmul itself writes the softmax denominator
  replicated across psum partitions 64..127 (a free partition-broadcast
  feeding a lane-parallel reciprocal+multiply normalize).  Diagonal
  chunks trim causally-dead columns from the score matmul, exp, and AV
  matmul; the final window normalizes and projects each 128-row column
  block as soon as its AV accumulation is final.
"""

import numpy as np
import ml_dtypes

BF16 = ml_dtypes.bfloat16

B, T, E = 2, 2048, 1024
H, HD = 16, 64
G = 4             # head groups (cores per batch)
HL = H // G       # heads per core
DL = HL * HD      # local qkv dim = 256
TC = 512          # T chunk (matmul moving free dim)
NJ = T // TC      # 4 q-windows
KC = 128          # k-chunk (contraction tile for attention)
NC_ = T // KC     # 16 k-chunks
SCALE = 1.0 / float(np.sqrt(HD))

_CACHE = {}


def _build_bass():
    import concourse.mybir as mybir
    import concourse.tile as tile
    from concourse import bacc

    f32 = mybir.dt.float32
    bf16 = mybir.dt.bfloat16
    EXP = mybir.ActivationFunctionType.Exp

    nc = bacc.Bacc("TRN2", target_bir_lowering=False, debug=False)
    # window-major contiguous layouts: one big DMA per tensor/window
    xtw_d = [nc.dram_tensor(f"xtw{j}", [128, 8 * TC], bf16,
                            kind="ExternalInput").ap() for j in range(NJ)]
    wqk_d = nc.dram_tensor("wqk", [128, 8 * 512], bf16, kind="ExternalInput").ap()
    wv_d = nc.dram_tensor("wv", [128, 8 * 256], bf16, kind="ExternalInput").ap()
    wo_d = nc.dram_tensor("wo", [DL, E], bf16, kind="ExternalInput").ap()
    cos_d = nc.dram_tensor("cosf", [128, T], bf16, kind="ExternalInput").ap()
    sin_d = nc.dram_tensor("sinf", [128, T], bf16, kind="ExternalInput").ap()
    rm_d = nc.dram_tensor("rmat", [128, 128], bf16, kind="ExternalInput").ap()
    y_d = nc.dram_tensor("y", [T, E], bf16, kind="ExternalOutput").ap()

    NKK = E // KC  # 8 contraction chunks for the projections

    with tile.TileContext(nc) as tc:
        with (
            tc.tile_pool(name="consts", bufs=1) as consts,
            tc.tile_pool(name="stp", bufs=2, space="PSUM") as stp,     # proj/scores/yp
            tc.tile_pool(name="avp", bufs=1, space="PSUM") as avp,     # v proj / AV
            tc.tile_pool(name="sbp", bufs=3) as sbp,                   # plain bf16
            tc.tile_pool(name="tmp_sb", bufs=3) as tmp_sb,             # rope tmps
            tc.tile_pool(name="est_sb", bufs=8) as est_sb,
            tc.tile_pool(name="attn_sb", bufs=2) as attn_sb,
            tc.tile_pool(name="ysb_p", bufs=2) as ysb_p,
            tc.tile_pool(name="small_sb", bufs=3) as small_sb,
        ):
            # ---- constant tiles (one big contiguous DMA each, ordered
            # by earliest first use; ~1MB transfers run near peak BW) ----
            wqk = consts.tile([128, 8 * 512], bf16, tag="wqk")
            wv = consts.tile([128, 8 * 256], bf16, tag="wv")
            xtw = [consts.tile([128, 8 * TC], bf16, tag=f"xtw{j}",
                               name=f"xtw{j}") for j in range(NJ)]
            cosf = consts.tile([128, T], bf16, tag="cosf")
            sinf = consts.tile([128, T], bf16, tag="sinf")
            rmat = consts.tile([128, 128], bf16, tag="rmat")

            HW = 4 * TC
            nc.sync.dma_start(out=rmat, in_=rm_d)
            nc.sync.dma_start(out=xtw[0][:, 0:HW], in_=xtw_d[0][:, 0:HW])
            nc.sync.dma_start(out=wqk[:, 0:HW], in_=wqk_d[:, 0:HW])
            nc.sync.dma_start(out=xtw[0][:, HW:2 * HW], in_=xtw_d[0][:, HW:2 * HW])
            nc.sync.dma_start(out=wqk[:, HW:2 * HW], in_=wqk_d[:, HW:2 * HW])
            nc.sync.dma_start(out=cosf[:, 0:TC], in_=cos_d[:, 0:TC])
            nc.sync.dma_start(out=sinf[:, 0:TC], in_=sin_d[:, 0:TC])
            nc.sync.dma_start(out=wv, in_=wv_d)
            nc.sync.dma_start(out=xtw[1], in_=xtw_d[1])
            nc.sync.dma_start(out=cosf[:, TC:2 * TC], in_=cos_d[:, TC:2 * TC])
            nc.sync.dma_start(out=sinf[:, TC:2 * TC], in_=sin_d[:, TC:2 * TC])
            wo = []
            for tau in range(2):
                t = consts.tile([128, E], bf16, tag=f"wo{tau}", name=f"wo{tau}")
                nc.sync.dma_start(out=t, in_=wo_d[tau * 128:(tau + 1) * 128, :])
                wo.append(t)
            nc.sync.dma_start(out=xtw[2], in_=xtw_d[2])
            nc.sync.dma_start(out=cosf[:, 2 * TC:3 * TC], in_=cos_d[:, 2 * TC:3 * TC])
            nc.sync.dma_start(out=sinf[:, 2 * TC:3 * TC], in_=sin_d[:, 2 * TC:3 * TC])
            nc.sync.dma_start(out=xtw[3], in_=xtw_d[3])
            nc.sync.dma_start(out=cosf[:, 3 * TC:], in_=cos_d[:, 3 * TC:])
            nc.sync.dma_start(out=sinf[:, 3 * TC:], in_=sin_d[:, 3 * TC:])

            def w_qk(kk, cc):
                """lhsT slice of the q/k weights: contraction rows kk*128,
                output columns cc..cc+128 of [q0 q1 k0 k1]."""
                return wqk[:, kk * 512 + cc:kk * 512 + cc + 128]

            def xt_s(c, kk):
                """[128, 128] xT block: T rows c*128.., contraction kk*128."""
                jj, tt = divmod(c, 4)
                return xtw[jj][:, kk * TC + tt * KC:kk * TC + (tt + 1) * KC]

            # triangular band mask: band[p, f] = 1 if f >= p else 0; stored
            # twice side by side so one strided mul covers both head
            # column-windows of an est tile
            band2 = consts.tile([128, 2 * KC], bf16, tag="band2")
            nc.gpsimd.memset(band2, 1.0)
            for wdup in range(2):
                nc.gpsimd.affine_select(
                    out=band2[:, wdup * KC:(wdup + 1) * KC],
                    in_=band2[:, wdup * KC:(wdup + 1) * KC],
                    compare_op=mybir.AluOpType.is_ge, fill=0.0,
                    base=0, pattern=[[1, KC]], channel_multiplier=-1,
                )

            # resident projection outputs (natural head-contiguous layout)
            qn = [[consts.tile([128, TC], bf16, tag=f"qn{tau}_{j}",
                               name=f"qn{tau}_{j}") for j in range(NJ)]
                  for tau in range(2)]
            kn = [[consts.tile([128, TC], bf16, tag=f"kn{tau}_{j}",
                               name=f"kn{tau}_{j}") for j in range(NJ)]
                  for tau in range(2)]
            # v tiles: per head 64 data columns + 64 ones columns, so the AV
            # matmul itself replicates the softmax denominator across psum
            # partitions 64..127 (free partition-broadcast)
            vsb = [consts.tile([128, HL * 128], bf16, tag=f"v{c}", name=f"v{c}")
                   for c in range(NC_)]
            for c in range(NC_):
                vv = vsb[c].rearrange("p (h d) -> p h d", h=HL)
                nc.gpsimd.memset(vv[:, :, 64:128], 1.0)

            # ---------------- projection emitters ----------------
            # The rotate-half permutation matmul of part p is deferred (it
            # depends on an ACT copy of part p's psum; the in-order PE
            # queue would stall on it if emitted immediately).
            rot_pend = []

            def flush_rot(keep=0):
                while len(rot_pend) > keep:
                    pr, sb, ta, tb, dstt, js = rot_pend.pop(0)
                    nc.tensor.matmul(pr, lhsT=rmat, rhs=sb,
                                     start=True, stop=True)
                    nc.vector.tensor_mul(ta, sb, cosf[:, js])
                    nc.vector.tensor_mul(tb, pr, sinf[:, js])
                    nc.vector.tensor_add(dstt, ta, tb)

            proj_pend = {}

            def emit_proj_qk(jj, base, tau, dst, half):
                """Half a plain projection (4 contraction chunks); the
                second half adds the bf16 copy + deferred rotate_half matmul
                + RoPE combine into dst[tau][jj]."""
                js = slice(jj * TC, (jj + 1) * TC)
                cc = base + 128 * tau
                key = (jj, base, tau)
                if half == 0:
                    p2 = stp.tile([128, 2 * TC], f32, tag="st",
                                  name=f"pp{base}_{tau}_{jj}")
                    proj_pend[key] = p2
                else:
                    p2 = proj_pend.pop(key)
                ps = p2[:, 0:TC]
                pr = p2[:, TC:2 * TC]
                for kk in range(4 * half, 4 * half + 4):
                    nc.tensor.matmul(
                        ps, lhsT=w_qk(kk, cc),
                        rhs=xtw[jj][:, kk * TC:(kk + 1) * TC],
                        start=(kk == 0), stop=(kk == NKK - 1))
                if half == 0:
                    return
                sb = sbp.tile([128, TC], bf16, tag="sb", name=f"sb{base}_{tau}_{jj}")
                nc.vector.tensor_copy(sb, ps)
                ta = tmp_sb.tile([128, TC], bf16, tag="ropeA",
                                 name=f"ra{base}_{tau}_{jj}")
                tb = tmp_sb.tile([128, TC], bf16, tag="ropeB",
                                 name=f"rb{base}_{tau}_{jj}")
                rot_pend.append((pr, sb, ta, tb, dst[tau][jj], js))
                flush_rot(keep=1)

            def emit_proj_v(jj, half):
                """v projection for T chunks [jj*4 + 2*half, +2)."""
                js0 = jj * (TC // KC) + 2 * half
                ps = stp.tile([128, 2 * TC], f32, tag="st",
                              name=f"pv{jj}_{half}")
                for tt in range(2):
                    c = js0 + tt
                    for kk in range(NKK):
                        nc.tensor.matmul(
                            ps[:, tt * DL:(tt + 1) * DL],
                            lhsT=xt_s(c, kk),
                            rhs=wv[:, kk * 256:(kk + 1) * 256],
                            start=(kk == 0), stop=(kk == NKK - 1))
                for tt in range(2):
                    c = js0 + tt
                    vv = vsb[c].rearrange("p (h d) -> p h d", h=HL)
                    src = ps[:, tt * DL:(tt + 1) * DL].rearrange(
                        "p (h d) -> p h d", h=HL)
                    nc.vector.tensor_copy(vv[:, :, 0:64], src)

            # ---------------- attention emitters ----------------
            ats = [None] * NJ
            av4s = [None] * NJ
            av_ests = [dict() for _ in range(NJ)]

            def emit_score(j, c):
                """Score matmuls + exp + diagonal band mask for chunk c."""
                d = c - 4 * j          # 0..3 on the diagonal
                coff = KC * d if d > 0 else 0
                sts = [stp.tile([128, 2 * TC], f32, tag="st",
                                name=f"st{j}_{c}_{i}") for i in range(2)]
                # one K=64 matmul per head; consecutive MMs use disjoint
                # row-groups of the PE array so they run concurrently
                for tau in range(2):
                    for ll in range(2):
                        h = 2 * tau + ll
                        stt = sts[h // 2]
                        w0_ = (h % 2) * TC
                        nc.tensor.matmul(
                            stt[:, w0_ + coff:w0_ + TC],
                            lhsT=kn[tau][c // 4][
                                64 * ll:64 * ll + 64,
                                (c % 4) * KC:(c % 4 + 1) * KC],
                            rhs=qn[tau][j][64 * ll:64 * ll + 64, coff:],
                            start=True, stop=True,
                            tile_position=(64 * ll, 0))
                ests = []
                for i in range(2):
                    est = est_sb.tile([128, 2 * TC], bf16, tag="est",
                                      name=f"est{j}_{c}_{i}")
                    if coff:
                        nc.scalar.activation(
                            est.rearrange("p (w c) -> p w c", w=2)[
                                :, :, coff:],
                            sts[i].rearrange("p (w c) -> p w c", w=2)[
                                :, :, coff:],
                            EXP, scale=SCALE)
                    else:
                        nc.scalar.activation(est, sts[i], EXP, scale=SCALE)
                    ests.append(est)
                if d >= 0:
                    b2 = band2.rearrange("p (w c) -> p w c", w=2)
                    for i in range(2):
                        ev = ests[i].rearrange("p (w c) -> p w c", w=2)[
                            :, :, KC * d:KC * (d + 1)]
                        nc.vector.tensor_mul(ev, ev, b2)
                av_ests[j][c] = ests

            def emit_av(j, c):
                """One AV accumulation step (4 heads) for window j chunk c."""
                nch = 4 * (j + 1)
                if c == 0:
                    av4s[j] = avp.tile([128, 4 * TC], f32, tag="av",
                                       name=f"av_{j}")
                av4 = av4s[j]
                coff = KC * (c - 4 * j) if c > 4 * j else 0
                for h in range(HL):
                    nc.tensor.matmul(
                        av4[0:128, h * TC + coff:(h + 1) * TC],
                        lhsT=vsb[c][:, 128 * h:128 * h + 128],
                        rhs=av_ests[j][c][h // 2][:, (h % 2) * TC + coff:
                                                  (h % 2 + 1) * TC],
                        start=(c == 0), stop=(c == nch - 1))

            def emit_y(jj, tt, cast_eng=None, warm_fill=False):
                """Output projection for q rows [jj*TC + tt*128, +128): full
                E columns, one contiguous DMA.  warm_fill runs two throwaway
                matmuls into the same psum tile first: they depend only on
                resident tiles, so they execute during the normalize-chain
                wait and keep the PE clock warm for the real matmuls (which
                overwrite them with start=True)."""
                at = ats[jj]
                yp = stp.tile([128, 2 * TC], f32, tag="st", name=f"yp{jj}_{tt}")
                if warm_fill:
                    for _ in range(2):
                        nc.tensor.matmul(
                            yp[:, 0:TC], lhsT=kn[1][3][:, 0:128],
                            rhs=qn[1][3][:, 0:TC], start=True, stop=True)
                for tau in range(2):
                    for n in range(2):
                        nc.tensor.matmul(
                            yp[:, n * TC:(n + 1) * TC],
                            lhsT=at[:, tau * TC + tt * KC:tau * TC + (tt + 1) * KC],
                            rhs=wo[tau][:, n * TC:(n + 1) * TC],
                            start=(tau == 0), stop=(tau == 1))
                ys = ysb_p.tile([128, 2 * TC], bf16, tag="y", name=f"ys{jj}_{tt}")
                if cast_eng is nc.scalar:
                    nc.scalar.copy(ys, yp)
                else:
                    nc.vector.tensor_copy(ys, yp)
                # output rides the sync HWDGE ring (input is done by the
                # time the first projection lands; keeping these off the
                # scalar queue protects the exp cadence from DMA-sem waits)
                nc.sync.dma_start(
                    out=y_d[jj * TC + tt * KC:jj * TC + (tt + 1) * KC, :],
                    in_=ys)

            def emit_norm(j, dc_scalar=False):
                """Softmax normalize window j: the AV matmul already
                replicated the denominator on psum partitions 64..127; copy
                it to a base-0 SBUF tile (reciprocal_approx requires that).
                Head pairs (0,2)/(1,3) occupy one row-half x full width of
                the merged at tile, so each pair is one copy/recip/mul."""
                at = ats[j]
                av4 = av4s[j]
                avh = av4.rearrange("p (g w) -> p g w", g=2)
                for l in range(2):     # l = row half = heads (l, l+2)
                    dc = small_sb.tile([64, 2 * TC], f32, tag=f"dc{l}",
                                       name=f"dc{j}_{l}")
                    if dc_scalar:
                        nc.scalar.copy(
                            dc.rearrange("p (g w) -> p g w", g=2),
                            avh[64:128, :, l * TC:(l + 1) * TC])
                    else:
                        nc.vector.tensor_copy(
                            dc.rearrange("p (g w) -> p g w", g=2),
                            avh[64:128, :, l * TC:(l + 1) * TC])
                    rd = small_sb.tile([64, 2 * TC], f32, tag=f"rd{l}",
                                       name=f"rd{j}_{l}")
                    nc.vector.reciprocal_approx_fast(out=rd, in_=dc)
                    nc.vector.tensor_mul(
                        at[64 * l:64 * l + 64, :].rearrange(
                            "p (g w) -> p g w", g=2),
                        avh[0:64, :, l * TC:(l + 1) * TC],
                        rd.rearrange("p (g w) -> p g w", g=2))

            def keep_warm(n=2, name="kw"):
                t = avp.tile([128, 4 * TC], f32, tag="av", name=name)
                for i in range(n):
                    nc.tensor.matmul(
                        t[:, (i % 2) * TC:(i % 2) * TC + TC],
                        lhsT=kn[1][3][:, 0:128], rhs=qn[1][3][:, 0:TC],
                        start=True, stop=True)

            # ---------------- prologue: window-0 projections ----------------
            keep_warm(16, name="kw_pro")
            for tau in range(2):
                emit_proj_qk(0, 0, tau, qn, 0)
                emit_proj_qk(0, DL, tau, kn, 0)
            for tau in range(2):
                emit_proj_qk(0, 0, tau, qn, 1)
                emit_proj_qk(0, DL, tau, kn, 1)
            emit_proj_v(0, 0)
            emit_proj_v(0, 1)
            flush_rot()

            # ---------------- pipelined window loop ----------------
            pre_scored = set()
            for j in range(NJ):
                nch = 4 * (j + 1)
                ats[j] = attn_sb.tile([128, 2 * TC], bf16, tag="attn",
                                      name=f"attn_{j}")
                flush_rot()

                side = {c: [] for c in range(nch)}
                if j > 0:
                    pn = 4 * j
                    side[0].append(lambda p=j - 1, c=pn - 2: emit_av(p, c))
                    side[0].append(lambda p=j - 1, c=pn - 1: emit_av(p, c))
                    side[1].append(lambda p=j - 1: emit_norm(p))
                    # previous window's output projections after normalize
                    ystep = max(1, (nch - 3) // 4)
                    for tt in range(4):
                        side[min(nch - 1, 3 + tt * ystep)].append(
                            lambda p=j - 1, t=tt: emit_y(p, t))
                    # own v projection (vsb[4j..4j+3] first used by the AV
                    # at chunk 4j+2): scheduled as late as its deadline
                    # allows to smooth the PE load across the window
                    side[max(0, 4 * j - 1)].append(
                        lambda p=j: emit_proj_v(p, 0))
                    side[4 * j].append(
                        lambda p=j: emit_proj_v(p, 1))
                if j + 1 < NJ:
                    # q/k projections for the next window
                    porder = [(0, 0), (DL, 0), (0, 1), (DL, 1)]
                    pstep = max(1, (nch - 2) // 4)
                    for i, (base, tau) in enumerate(porder):
                        dst = qn if base == 0 else kn
                        side[min(nch - 2, i * pstep)].append(
                            lambda b=base, t=tau, d=dst, p=j + 1:
                            emit_proj_qk(p, b, t, d, 0))
                        side[min(nch - 2, i * pstep)].append(
                            lambda b=base, t=tau, d=dst, p=j + 1:
                            emit_proj_qk(p, b, t, d, 1))
                    # prefetch the next window's first score chunk at this
                    # window's last chunk: its exp bridges the boundary gap
                    # on the scalar engine (placed only after window j's own
                    # last score so the psum rotation order is preserved)
                    def pre_score(p=j + 1):
                        flush_rot()     # qn/kn[p] combines must be emitted
                        emit_score(p, 0)
                        emit_score(p, 1)
                        pre_scored.add((p, 0))
                        pre_scored.add((p, 1))
                    side[nch - 1].append(pre_score)

                for c in range(nch):
                    if (j, c) not in pre_scored:
                        emit_score(j, c)
                    for t in side[c]:
                        t()
                    # flush any pending rotate-half matmul now: deferring it
                    # to the next part would hold its psum tile across
                    # chunks, and the 2-buffer rotation would make upcoming
                    # score matmuls wait on that release
                    flush_rot()
                    if c >= 2:
                        emit_av(j, c - 2)

            # ---------------- tail: window 3 ----------------
            # column block tt of the AV psum is final once av(j, 12+tt) has
            # run (later diagonal chunks only touch columns beyond it), so
            # each 128-q block normalizes + projects while later chunks are
            # still accumulating -- the PE never idles long enough to lose
            # its HAM warm state
            j = NJ - 1
            nch = 4 * NJ

            def tail_tt(tt):
                at = ats[j]
                av4 = av4s[j]
                avh = av4.rearrange("p (g w) -> p g w", g=2)
                dc = small_sb.tile([64, HL * KC], f32, tag="dct",
                                   name=f"dct{tt}")
                nc.scalar.copy(
                    dc.rearrange("p (h c) -> p h c", h=HL),
                    av4.rearrange("p (h c) -> p h c", h=HL)[
                        64:128, :, tt * KC:(tt + 1) * KC])
                rd = small_sb.tile([64, HL * KC], f32, tag="rdt",
                                   name=f"rdt{tt}")
                nc.vector.reciprocal_approx_fast(out=rd, in_=dc)
                rdh = rd.rearrange("p (g l c) -> p l g c", g=2, l=2)
                ath = at.rearrange("p (g w) -> p g w", g=2)
                for l in range(2):     # heads (l, l+2)
                    nc.vector.tensor_mul(
                        ath[64 * l:64 * l + 64, :,
                            tt * KC:(tt + 1) * KC],
                        avh[0:64, :,
                            l * TC + tt * KC:l * TC + (tt + 1) * KC],
                        rdh[:, l, :, :])
                emit_y(j, tt, cast_eng=nc.scalar)

            tail_tt(0)
            emit_av(j, nch - 2)
            tail_tt(1)
            emit_av(j, nch - 1)
            tail_tt(2)
            tail_tt(3)

    nc.compile()
    return nc


def _host_inputs(x, cos, sin, w_qkv, w_out):
    """Shard + lay out the full inputs for the 8 cores."""
    # natural-layout tables: row 64*l + d = cos/sin[t, d]
    cosf = np.ascontiguousarray(np.tile(cos.T, (2, 1))).astype(BF16)
    sinf = np.ascontiguousarray(np.tile(sin.T, (2, 1))).astype(BF16)

    # xtw[b][jj] = [128, 8*TC]: kk-major blocks of the T-window jj of x^T
    xts = [x[b].T.astype(BF16) for b in range(B)]
    xtws = []
    for b in range(B):
        xt = xts[b].reshape(8, KC, NJ, TC)           # (kk, p, jj, t)
        xtws.append([np.ascontiguousarray(
            xt[:, :, jj, :].transpose(1, 0, 2).reshape(KC, 8 * TC))
            for jj in range(NJ)])

    # lhsT for the rotate_half permutation matmul: rot = rmat.T @ q with
    # rot[d] = -q[d+32] (d<32), q[d-32] (d>=32) per 64-row head block
    r64 = np.zeros((64, 64), dtype=np.float32)
    r64[np.arange(32) + 32, np.arange(32)] = -1.0
    r64[np.arange(32), np.arange(32) + 32] = 1.0
    rmat = np.zeros((128, 128), dtype=np.float32)
    rmat[0:64, 0:64] = r64
    rmat[64:128, 64:128] = r64
    rmat = rmat.astype(BF16)

    in_maps = []
    for core in range(8):
        b, g = divmod(core, G)
        qblk = w_qkv[:, G * g * HD:(G * g + HL) * HD]
        kblk = w_qkv[:, E + G * g * HD:E + (G * g + HL) * HD]
        vblk = w_qkv[:, 2 * E + DL * g:2 * E + DL * (g + 1)]
        # wqk: kk-major blocks of [q0 q1 k0 k1] (512 cols each)
        qk = np.concatenate([qblk, kblk], axis=1).astype(BF16)  # (E, 512)
        wqkl = np.ascontiguousarray(
            qk.reshape(8, KC, 512).transpose(1, 0, 2).reshape(KC, 8 * 512))
        # wv: kk-major blocks of the 256 v columns
        wvl = np.ascontiguousarray(
            vblk.astype(BF16).reshape(8, KC, 256).transpose(1, 0, 2)
            .reshape(KC, 8 * 256))
        wol = np.ascontiguousarray(w_out[DL * g:DL * (g + 1), :]).astype(BF16)
        m = {"wqk": wqkl, "wv": wvl, "wo": wol, "cosf": cosf, "sinf": sinf,
             "rmat": rmat}
        for jj in range(NJ):
            m[f"xtw{jj}"] = xtws[b][jj]
        in_maps.append(m)
    return in_maps


def kernel(x, cos, sin, w_qkv, w_out):
    from concourse import bass_utils

    if "nc" not in _CACHE:
        _CACHE["nc"] = _build_bass()
    nc = _CACHE["nc"]

    in_maps = _host_inputs(
        np.asarray(x, dtype=np.float32), np.asarray(cos, dtype=np.float32),
        np.asarray(sin, dtype=np.float32), np.asarray(w_qkv, dtype=np.float32),
        np.asarray(w_out, dtype=np.float32))

    res = bass_utils.run_bass_kernel_spmd(nc, in_maps, core_ids=list(range(8)))

    y = np.zeros((B, T, E), dtype=np.float32)
    for core in range(8):
        b = core // G
        y[b] += res.results[core]["y"].astype(np.float32)
    return y


# revision 62
# speedup vs baseline: 1.0308x; 1.0308x over previous
"""Causal self-attention (B=2, T=2048, E=1024, H=16) on 8 trn2 NeuronCores.

Sharding: core = b*4 + g  (b = batch index, g = head-group of 4 heads).
Each core computes its 4 heads' attention for its batch plus a partial
output projection; the host sums the 4 partials per batch.

v3 structure (single software-pipelined loop):
  All inputs arrive as a few ~0.5-1MB contiguous DMAs on the sync HWDGE
  ring, ordered by first use (host pre-lays x^T window-major / kk-major
  so every transfer is one dense block); outputs also use the sync ring,
  which is idle by the time the first result is ready (keeping them off
  the scalar queue protects the exp cadence from DMA-semaphore waits).
  The prologue computes only window-0 projections (v + q/k with RoPE via
  a block-diag permutation matmul).  The attention loop (windows j=0..3,
  k-chunks c) is paced by the ACT exp cadence; everything else rides in
  its slack as side tasks with deadlines: q/k projections for window
  j+1, window j's own v projection, output projections for window j-1,
  and the trailing AV matmuls + softmax-normalize of window j-1 (moved
  into window j's first chunks so next-window scores never wait on
  them).  Scores are computed transposed (S^T = K Q^T) with two heads
  row-packed per matmul; each head's V tile carries 64 appended ones
  columns so the AV matmul itself writes the softmax denominator
  replicated across psum partitions 64..127 (a free partition-broadcast
  feeding a lane-parallel reciprocal+multiply normalize).  Diagonal
  chunks trim causally-dead columns from the score matmul, exp, and AV
  matmul; the final window normalizes and projects each 128-row column
  block as soon as its AV accumulation is final.
"""

import numpy as np
import ml_dtypes

BF16 = ml_dtypes.bfloat16

B, T, E = 2, 2048, 1024
H, HD = 16, 64
G = 4             # head groups (cores per batch)
HL = H // G       # heads per core
DL = HL * HD      # local qkv dim = 256
TC = 512          # T chunk (matmul moving free dim)
NJ = T // TC      # 4 q-windows
KC = 128          # k-chunk (contraction tile for attention)
NC_ = T // KC     # 16 k-chunks
SCALE = 1.0 / float(np.sqrt(HD))

_CACHE = {}


def _build_bass():
    import concourse.mybir as mybir
    import concourse.tile as tile
    from concourse import bacc

    f32 = mybir.dt.float32
    bf16 = mybir.dt.bfloat16
    EXP = mybir.ActivationFunctionType.Exp

    nc = bacc.Bacc("TRN2", target_bir_lowering=False, debug=False)
    # window-major contiguous layouts: one big DMA per tensor/window
    xtw_d = [nc.dram_tensor(f"xtw{j}", [128, 8 * TC], bf16,
                            kind="ExternalInput").ap() for j in range(NJ)]
    wqk_d = nc.dram_tensor("wqk", [128, 8 * 512], bf16, kind="ExternalInput").ap()
    wv_d = nc.dram_tensor("wv", [128, 8 * 256], bf16, kind="ExternalInput").ap()
    wo_d = nc.dram_tensor("wo", [DL, E], bf16, kind="ExternalInput").ap()
    cos_d = nc.dram_tensor("cosf", [128, T], bf16, kind="ExternalInput").ap()
    sin_d = nc.dram_tensor("sinf", [128, T], bf16, kind="ExternalInput").ap()
    rm_d = nc.dram_tensor("rmat", [128, 128], bf16, kind="ExternalInput").ap()
    y_d = nc.dram_tensor("y", [T, E], bf16, kind="ExternalOutput").ap()

    NKK = E // KC  # 8 contraction chunks for the projections

    with tile.TileContext(nc) as tc:
        with (
            tc.tile_pool(name="consts", bufs=1) as consts,
            tc.tile_pool(name="stp", bufs=2, space="PSUM") as stp,     # proj/scores/yp
            tc.tile_pool(name="avp", bufs=1, space="PSUM") as avp,     # v proj / AV
            tc.tile_pool(name="sbp", bufs=3) as sbp,                   # plain bf16
            tc.tile_pool(name="tmp_sb", bufs=3) as tmp_sb,             # rope tmps
            tc.tile_pool(name="est_sb", bufs=8) as est_sb,
            tc.tile_pool(name="attn_sb", bufs=2) as attn_sb,
            tc.tile_pool(name="ysb_p", bufs=2) as ysb_p,
            tc.tile_pool(name="small_sb", bufs=3) as small_sb,
        ):
            # ---- constant tiles (one big contiguous DMA each, ordered
            # by earliest first use; ~1MB transfers run near peak BW) ----
            wqk = consts.tile([128, 8 * 512], bf16, tag="wqk")
            wv = consts.tile([128, 8 * 256], bf16, tag="wv")
            xtw = [consts.tile([128, 8 * TC], bf16, tag=f"xtw{j}",
                               name=f"xtw{j}") for j in range(NJ)]
            cosf = consts.tile([128, T], bf16, tag="cosf")
            sinf = consts.tile([128, T], bf16, tag="sinf")
            rmat = consts.tile([128, 128], bf16, tag="rmat")

            HW = 4 * TC
            nc.sync.dma_start(out=rmat, in_=rm_d)
            nc.sync.dma_start(out=xtw[0][:, 0:HW], in_=xtw_d[0][:, 0:HW])
            nc.sync.dma_start(out=wqk[:, 0:HW], in_=wqk_d[:, 0:HW])
            nc.sync.dma_start(out=xtw[0][:, HW:2 * HW], in_=xtw_d[0][:, HW:2 * HW])
            nc.sync.dma_start(out=wqk[:, HW:2 * HW], in_=wqk_d[:, HW:2 * HW])
            nc.sync.dma_start(out=cosf[:, 0:TC], in_=cos_d[:, 0:TC])
            nc.sync.dma_start(out=sinf[:, 0:TC], in_=sin_d[:, 0:TC])
            nc.sync.dma_start(out=wv, in_=wv_d)
            nc.sync.dma_start(out=xtw[1], in_=xtw_d[1])
            nc.sync.dma_start(out=cosf[:, TC:2 * TC], in_=cos_d[:, TC:2 * TC])
            nc.sync.dma_start(out=sinf[:, TC:2 * TC], in_=sin_d[:, TC:2 * TC])
            wo = []
            for tau in range(2):
                t = consts.tile([128, E], bf16, tag=f"wo{tau}", name=f"wo{tau}")
                nc.sync.dma_start(out=t, in_=wo_d[tau * 128:(tau + 1) * 128, :])
                wo.append(t)
            nc.sync.dma_start(out=xtw[2], in_=xtw_d[2])
            nc.sync.dma_start(out=cosf[:, 2 * TC:3 * TC], in_=cos_d[:, 2 * TC:3 * TC])
            nc.sync.dma_start(out=sinf[:, 2 * TC:3 * TC], in_=sin_d[:, 2 * TC:3 * TC])
            nc.sync.dma_start(out=xtw[3], in_=xtw_d[3])
            nc.sync.dma_start(out=cosf[:, 3 * TC:], in_=cos_d[:, 3 * TC:])
            nc.sync.dma_start(out=sinf[:, 3 * TC:], in_=sin_d[:, 3 * TC:])

            def w_qk(kk, cc):
                """lhsT slice of the q/k weights: contraction rows kk*128,
                output columns cc..cc+128 of [q0 q1 k0 k1]."""
                return wqk[:, kk * 512 + cc:kk * 512 + cc + 128]

            def xt_s(c, kk):
                """[128, 128] xT block: T rows c*128.., contraction kk*128."""
                jj, tt = divmod(c, 4)
                return xtw[jj][:, kk * TC + tt * KC:kk * TC + (tt + 1) * KC]

            # triangular band mask: band[p, f] = 1 if f >= p else 0; stored
            # twice side by side so one strided mul covers both head
            # column-windows of an est tile
            band2 = consts.tile([128, 2 * KC], bf16, tag="band2")
            nc.gpsimd.memset(band2, 1.0)
            for wdup in range(2):
                nc.gpsimd.affine_select(
                    out=band2[:, wdup * KC:(wdup + 1) * KC],
                    in_=band2[:, wdup * KC:(wdup + 1) * KC],
                    compare_op=mybir.AluOpType.is_ge, fill=0.0,
                    base=0, pattern=[[1, KC]], channel_multiplier=-1,
                )

            # resident projection outputs (natural head-contiguous layout)
            qn = [[consts.tile([128, TC], bf16, tag=f"qn{tau}_{j}",
                               name=f"qn{tau}_{j}") for j in range(NJ)]
                  for tau in range(2)]
            kn = [[consts.tile([128, TC], bf16, tag=f"kn{tau}_{j}",
                               name=f"kn{tau}_{j}") for j in range(NJ)]
                  for tau in range(2)]
            # v tiles: per head 64 data columns + 64 ones columns, so the AV
            # matmul itself replicates the softmax denominator across psum
            # partitions 64..127 (free partition-broadcast)
            vsb = [consts.tile([128, HL * 128], bf16, tag=f"v{c}", name=f"v{c}")
                   for c in range(NC_)]
            for c in range(NC_):
                vv = vsb[c].rearrange("p (h d) -> p h d", h=HL)
                nc.gpsimd.memset(vv[:, :, 64:128], 1.0)

            # ---------------- projection emitters ----------------
            # The rotate-half permutation matmul of part p is deferred (it
            # depends on an ACT copy of part p's psum; the in-order PE
            # queue would stall on it if emitted immediately).
            rot_pend = []

            def flush_rot(keep=0):
                while len(rot_pend) > keep:
                    pr, sb, ta, tb, dstt, js = rot_pend.pop(0)
                    nc.tensor.matmul(pr, lhsT=rmat, rhs=sb,
                                     start=True, stop=True)
                    nc.vector.tensor_mul(ta, sb, cosf[:, js])
                    nc.vector.tensor_mul(tb, pr, sinf[:, js])
                    nc.vector.tensor_add(dstt, ta, tb)

            proj_pend = {}

            def emit_proj_qk(jj, base, tau, dst, half):
                """Half a plain projection (4 contraction chunks); the
                second half adds the bf16 copy + deferred rotate_half matmul
                + RoPE combine into dst[tau][jj]."""
                js = slice(jj * TC, (jj + 1) * TC)
                cc = base + 128 * tau
                key = (jj, base, tau)
                if half == 0:
                    p2 = stp.tile([128, 2 * TC], f32, tag="st",
                                  name=f"pp{base}_{tau}_{jj}")
                    proj_pend[key] = p2
                else:
                    p2 = proj_pend.pop(key)
                ps = p2[:, 0:TC]
                pr = p2[:, TC:2 * TC]
                for kk in range(4 * half, 4 * half + 4):
                    nc.tensor.matmul(
                        ps, lhsT=w_qk(kk, cc),
                        rhs=xtw[jj][:, kk * TC:(kk + 1) * TC],
                        start=(kk == 0), stop=(kk == NKK - 1))
                if half == 0:
                    return
                sb = sbp.tile([128, TC], bf16, tag="sb", name=f"sb{base}_{tau}_{jj}")
                nc.vector.tensor_copy(sb, ps)
                ta = tmp_sb.tile([128, TC], bf16, tag="ropeA",
                                 name=f"ra{base}_{tau}_{jj}")
                tb = tmp_sb.tile([128, TC], bf16, tag="ropeB",
                                 name=f"rb{base}_{tau}_{jj}")
                rot_pend.append((pr, sb, ta, tb, dst[tau][jj], js))
                flush_rot(keep=1)

            def emit_proj_v(jj, half):
                """v projection for T chunks [jj*4 + 2*half, +2)."""
                js0 = jj * (TC // KC) + 2 * half
                ps = stp.tile([128, 2 * TC], f32, tag="st",
                              name=f"pv{jj}_{half}")
                for tt in range(2):
                    c = js0 + tt
                    for kk in range(NKK):
                        nc.tensor.matmul(
                            ps[:, tt * DL:(tt + 1) * DL],
                            lhsT=xt_s(c, kk),
                            rhs=wv[:, kk * 256:(kk + 1) * 256],
                            start=(kk == 0), stop=(kk == NKK - 1))
                for tt in range(2):
                    c = js0 + tt
                    vv = vsb[c].rearrange("p (h d) -> p h d", h=HL)
                    src = ps[:, tt * DL:(tt + 1) * DL].rearrange(
                        "p (h d) -> p h d", h=HL)
                    nc.vector.tensor_copy(vv[:, :, 0:64], src)

            # ---------------- attention emitters ----------------
            ats = [None] * NJ
            av4s = [None] * NJ
            av_ests = [dict() for _ in range(NJ)]

            def emit_score(j, c):
                """Score matmuls + exp + diagonal band mask for chunk c."""
                d = c - 4 * j          # 0..3 on the diagonal
                coff = KC * d if d > 0 else 0
                sts = [stp.tile([128, 2 * TC], f32, tag="st",
                                name=f"st{j}_{c}_{i}") for i in range(2)]
                # one K=64 matmul per head; consecutive MMs use disjoint
                # row-groups of the PE array so they run concurrently
                for tau in range(2):
                    for ll in range(2):
                        h = 2 * tau + ll
                        stt = sts[h // 2]
                        w0_ = (h % 2) * TC
                        nc.tensor.matmul(
                            stt[:, w0_ + coff:w0_ + TC],
                            lhsT=kn[tau][c // 4][
                                64 * ll:64 * ll + 64,
                                (c % 4) * KC:(c % 4 + 1) * KC],
                            rhs=qn[tau][j][64 * ll:64 * ll + 64, coff:],
                            start=True, stop=True,
                            tile_position=(64 * ll, 0))
                ests = []
                for i in range(2):
                    est = est_sb.tile([128, 2 * TC], bf16, tag="est",
                                      name=f"est{j}_{c}_{i}")
                    if coff:
                        nc.scalar.activation(
                            est.rearrange("p (w c) -> p w c", w=2)[
                                :, :, coff:],
                            sts[i].rearrange("p (w c) -> p w c", w=2)[
                                :, :, coff:],
                            EXP, scale=SCALE)
                    else:
                        nc.scalar.activation(est, sts[i], EXP, scale=SCALE)
                    ests.append(est)
                if d >= 0:
                    b2 = band2.rearrange("p (w c) -> p w c", w=2)
                    for i in range(2):
                        ev = ests[i].rearrange("p (w c) -> p w c", w=2)[
                            :, :, KC * d:KC * (d + 1)]
                        nc.vector.tensor_mul(ev, ev, b2)
                av_ests[j][c] = ests

            def emit_av(j, c):
                """One AV accumulation step (4 heads) for window j chunk c."""
                nch = 4 * (j + 1)
                if c == 0:
                    av4s[j] = avp.tile([128, 4 * TC], f32, tag="av",
                                       name=f"av_{j}")
                av4 = av4s[j]
                coff = KC * (c - 4 * j) if c > 4 * j else 0
                for h in range(HL):
                    nc.tensor.matmul(
                        av4[0:128, h * TC + coff:(h + 1) * TC],
                        lhsT=vsb[c][:, 128 * h:128 * h + 128],
                        rhs=av_ests[j][c][h // 2][:, (h % 2) * TC + coff:
                                                  (h % 2 + 1) * TC],
                        start=(c == 0), stop=(c == nch - 1))

            def emit_y(jj, tt, cast_eng=None, warm_fill=False):
                """Output projection for q rows [jj*TC + tt*128, +128): full
                E columns, one contiguous DMA.  warm_fill runs two throwaway
                matmuls into the same psum tile first: they depend only on
                resident tiles, so they execute during the normalize-chain
                wait and keep the PE clock warm for the real matmuls (which
                overwrite them with start=True)."""
                at = ats[jj]
                yp = stp.tile([128, 2 * TC], f32, tag="st", name=f"yp{jj}_{tt}")
                if warm_fill:
                    for _ in range(2):
                        nc.tensor.matmul(
                            yp[:, 0:TC], lhsT=kn[1][3][:, 0:128],
                            rhs=qn[1][3][:, 0:TC], start=True, stop=True)
                for tau in range(2):
                    for n in range(2):
                        nc.tensor.matmul(
                            yp[:, n * TC:(n + 1) * TC],
                            lhsT=at[:, tau * TC + tt * KC:tau * TC + (tt + 1) * KC],
                            rhs=wo[tau][:, n * TC:(n + 1) * TC],
                            start=(tau == 0), stop=(tau == 1))
                ys = ysb_p.tile([128, 2 * TC], bf16, tag="y", name=f"ys{jj}_{tt}")
                if cast_eng is nc.scalar:
                    nc.scalar.copy(ys, yp)
                else:
                    nc.vector.tensor_copy(ys, yp)
                # output rides the sync HWDGE ring (input is done by the
                # time the first projection lands; keeping these off the
                # scalar queue protects the exp cadence from DMA-sem waits)
                nc.sync.dma_start(
                    out=y_d[jj * TC + tt * KC:jj * TC + (tt + 1) * KC, :],
                    in_=ys)

            def emit_norm(j, dc_scalar=False):
                """Softmax normalize window j: the AV matmul already
                replicated the denominator on psum partitions 64..127; copy
                it to a base-0 SBUF tile (reciprocal_approx requires that).
                Head pairs (0,2)/(1,3) occupy one row-half x full width of
                the merged at tile, so each pair is one copy/recip/mul."""
                at = ats[j]
                av4 = av4s[j]
                avh = av4.rearrange("p (g w) -> p g w", g=2)
                for l in range(2):     # l = row half = heads (l, l+2)
                    dc = small_sb.tile([64, 2 * TC], f32, tag=f"dc{l}",
                                       name=f"dc{j}_{l}")
                    if dc_scalar:
                        nc.scalar.copy(
                            dc.rearrange("p (g w) -> p g w", g=2),
                            avh[64:128, :, l * TC:(l + 1) * TC])
                    else:
                        nc.vector.tensor_copy(
                            dc.rearrange("p (g w) -> p g w", g=2),
                            avh[64:128, :, l * TC:(l + 1) * TC])
                    rd = small_sb.tile([64, 2 * TC], f32, tag=f"rd{l}",
                                       name=f"rd{j}_{l}")
                    nc.vector.reciprocal_approx_fast(out=rd, in_=dc)
                    nc.vector.tensor_mul(
                        at[64 * l:64 * l + 64, :].rearrange(
                            "p (g w) -> p g w", g=2),
                        avh[0:64, :, l * TC:(l + 1) * TC],
                        rd.rearrange("p (g w) -> p g w", g=2))

            def keep_warm(n=2, name="kw"):
                t = avp.tile([128, 4 * TC], f32, tag="av", name=name)
                for i in range(n):
                    nc.tensor.matmul(
                        t[:, (i % 2) * TC:(i % 2) * TC + TC],
                        lhsT=kn[1][3][:, 0:128], rhs=qn[1][3][:, 0:TC],
                        start=True, stop=True)

            # ---------------- prologue: window-0 projections ----------------
            keep_warm(16, name="kw_pro")
            for tau in range(2):
                emit_proj_qk(0, 0, tau, qn, 0)
                emit_proj_qk(0, DL, tau, kn, 0)
            for tau in range(2):
                emit_proj_qk(0, 0, tau, qn, 1)
                emit_proj_qk(0, DL, tau, kn, 1)
            emit_proj_v(0, 0)
            emit_proj_v(0, 1)
            flush_rot()

            # ---------------- pipelined window loop ----------------
            pre_scored = set()
            for j in range(NJ):
                nch = 4 * (j + 1)
                ats[j] = attn_sb.tile([128, 2 * TC], bf16, tag="attn",
                                      name=f"attn_{j}")
                flush_rot()

                side = {c: [] for c in range(nch)}
                if j > 0:
                    pn = 4 * j
                    side[0].append(lambda p=j - 1, c=pn - 2: emit_av(p, c))
                    side[0].append(lambda p=j - 1, c=pn - 1: emit_av(p, c))
                    side[1].append(lambda p=j - 1: emit_norm(p))
                    # previous window's output projections after normalize
                    ystep = max(1, (nch - 3) // 4)
                    for tt in range(4):
                        side[min(nch - 1, 3 + tt * ystep)].append(
                            lambda p=j - 1, t=tt: emit_y(p, t))
                    # own v projection (vsb[4j..4j+3] first used by the AV
                    # at chunk 4j+2): scheduled as late as its deadline
                    # allows to smooth the PE load across the window
                    side[max(0, 4 * j - 1)].append(
                        lambda p=j: emit_proj_v(p, 0))
                    side[4 * j].append(
                        lambda p=j: emit_proj_v(p, 1))
                if j + 1 < NJ:
                    # q/k projections for the next window
                    porder = [(0, 0), (DL, 0), (0, 1), (DL, 1)]
                    pstep = max(1, (nch - 2) // 4)
                    for i, (base, tau) in enumerate(porder):
                        dst = qn if base == 0 else kn
                        side[min(nch - 2, i * pstep)].append(
                            lambda b=base, t=tau, d=dst, p=j + 1:
                            emit_proj_qk(p, b, t, d, 0))
                        side[min(nch - 2, i * pstep)].append(
                            lambda b=base, t=tau, d=dst, p=j + 1:
                            emit_proj_qk(p, b, t, d, 1))
                    # prefetch the next window's first score chunk at this
                    # window's last chunk: its exp bridges the boundary gap
                    # on the scalar engine (placed only after window j's own
                    # last score so the psum rotation order is preserved)
                    def pre_score(p=j + 1):
                        flush_rot()     # qn/kn[p] combines must be emitted
                        emit_score(p, 0)
                        emit_score(p, 1)
                        pre_scored.add((p, 0))
                        pre_scored.add((p, 1))
                    side[nch - 1].append(pre_score)

                for c in range(nch):
                    if (j, c) not in pre_scored:
                        emit_score(j, c)
                    for t in side[c]:
                        t()
                    # flush any pending rotate-half matmul now: deferring it
                    # to the next part would hold its psum tile across
                    # chunks, and the 2-buffer rotation would make upcoming
                    # score matmuls wait on that release
                    flush_rot()
                    if c >= 2:
                        emit_av(j, c - 2)

            # ---------------- tail: window 3 ----------------
            # column block tt of the AV psum is final once av(j, 12+tt) has
            # run (later diagonal chunks only touch columns beyond it), so
            # each 128-q block normalizes + projects while later chunks are
            # still accumulating -- the PE never idles long enough to lose
            # its HAM warm state
            j = NJ - 1
            nch = 4 * NJ

            def tail_tt(tt):
                at = ats[j]
                av4 = av4s[j]
                avh = av4.rearrange("p (g w) -> p g w", g=2)
                dc = small_sb.tile([64, HL * KC], f32, tag="dct",
                                   name=f"dct{tt}")
                nc.scalar.copy(
                    dc.rearrange("p (h c) -> p h c", h=HL),
                    av4.rearrange("p (h c) -> p h c", h=HL)[
                        64:128, :, tt * KC:(tt + 1) * KC])
                rd = small_sb.tile([64, HL * KC], f32, tag="rdt",
                                   name=f"rdt{tt}")
                nc.vector.reciprocal_approx_fast(out=rd, in_=dc)
                rdh = rd.rearrange("p (g l c) -> p l g c", g=2, l=2)
                ath = at.rearrange("p (g w) -> p g w", g=2)
                for l in range(2):     # heads (l, l+2)
                    nc.vector.tensor_mul(
                        ath[64 * l:64 * l + 64, :,
                            tt * KC:(tt + 1) * KC],
                        avh[0:64, :,
                            l * TC + tt * KC:l * TC + (tt + 1) * KC],
                        rdh[:, l, :, :])
                emit_y(j, tt, cast_eng=nc.scalar)

            emit_av(j, nch - 2)
            tail_tt(0)
            emit_av(j, nch - 1)
            tail_tt(1)
            tail_tt(2)
            tail_tt(3)

    nc.compile()
    return nc


def _host_inputs(x, cos, sin, w_qkv, w_out):
    """Shard + lay out the full inputs for the 8 cores."""
    # natural-layout tables: row 64*l + d = cos/sin[t, d]
    cosf = np.ascontiguousarray(np.tile(cos.T, (2, 1))).astype(BF16)
    sinf = np.ascontiguousarray(np.tile(sin.T, (2, 1))).astype(BF16)

    # xtw[b][jj] = [128, 8*TC]: kk-major blocks of the T-window jj of x^T
    xts = [x[b].T.astype(BF16) for b in range(B)]
    xtws = []
    for b in range(B):
        xt = xts[b].reshape(8, KC, NJ, TC)           # (kk, p, jj, t)
        xtws.append([np.ascontiguousarray(
            xt[:, :, jj, :].transpose(1, 0, 2).reshape(KC, 8 * TC))
            for jj in range(NJ)])

    # lhsT for the rotate_half permutation matmul: rot = rmat.T @ q with
    # rot[d] = -q[d+32] (d<32), q[d-32] (d>=32) per 64-row head block
    r64 = np.zeros((64, 64), dtype=np.float32)
    r64[np.arange(32) + 32, np.arange(32)] = -1.0
    r64[np.arange(32), np.arange(32) + 32] = 1.0
    rmat = np.zeros((128, 128), dtype=np.float32)
    rmat[0:64, 0:64] = r64
    rmat[64:128, 64:128] = r64
    rmat = rmat.astype(BF16)

    in_maps = []
    for core in range(8):
        b, g = divmod(core, G)
        qblk = w_qkv[:, G * g * HD:(G * g + HL) * HD]
        kblk = w_qkv[:, E + G * g * HD:E + (G * g + HL) * HD]
        vblk = w_qkv[:, 2 * E + DL * g:2 * E + DL * (g + 1)]
        # wqk: kk-major blocks of [q0 q1 k0 k1] (512 cols each)
        qk = np.concatenate([qblk, kblk], axis=1).astype(BF16)  # (E, 512)
        wqkl = np.ascontiguousarray(
            qk.reshape(8, KC, 512).transpose(1, 0, 2).reshape(KC, 8 * 512))
        # wv: kk-major blocks of the 256 v columns
        wvl = np.ascontiguousarray(
            vblk.astype(BF16).reshape(8, KC, 256).transpose(1, 0, 2)
            .reshape(KC, 8 * 256))
        wol = np.ascontiguousarray(w_out[DL * g:DL * (g + 1), :]).astype(BF16)
        m = {"wqk": wqkl, "wv": wvl, "wo": wol, "cosf": cosf, "sinf": sinf,
             "rmat": rmat}
        for jj in range(NJ):
            m[f"xtw{jj}"] = xtws[b][jj]
        in_maps.append(m)
    return in_maps


def kernel(x, cos, sin, w_qkv, w_out):
    from concourse import bass_utils

    if "nc" not in _CACHE:
        _CACHE["nc"] = _build_bass()
    nc = _CACHE["nc"]

    in_maps = _host_inputs(
        np.asarray(x, dtype=np.float32), np.asarray(cos, dtype=np.float32),
        np.asarray(sin, dtype=np.float32), np.asarray(w_qkv, dtype=np.float32),
        np.asarray(w_out, dtype=np.float32))

    res = bass_utils.run_bass_kernel_spmd(nc, in_maps, core_ids=list(range(8)))

    y = np.zeros((B, T, E), dtype=np.float32)
    for core in range(8):
        b = core // G
        y[b] += res.results[core]["y"].astype(np.float32)
    return y


# revision 63
# speedup vs baseline: 1.0389x; 1.0078x over previous
"""Causal self-attention (B=2, T=2048, E=1024, H=16) on 8 trn2 NeuronCores.

Sharding: core = b*4 + g  (b = batch index, g = head-group of 4 heads).
Each core computes its 4 heads' attention for its batch plus a partial
output projection; the host sums the 4 partials per batch.

v3 structure (single software-pipelined loop):
  All inputs arrive as a few ~0.5-1MB contiguous DMAs on the sync HWDGE
  ring, ordered by first use (host pre-lays x^T window-major / kk-major
  so every transfer is one dense block); outputs also use the sync ring,
  which is idle by the time the first result is ready (keeping them off
  the scalar queue protects the exp cadence from DMA-semaphore waits).
  The prologue computes only window-0 projections (v + q/k with RoPE via
  a block-diag permutation matmul).  The attention loop (windows j=0..3,
  k-chunks c) is paced by the ACT exp cadence; everything else rides in
  its slack as side tasks with deadlines: q/k projections for window
  j+1, window j's own v projection, output projections for window j-1,
  and the trailing AV matmuls + softmax-normalize of window j-1 (moved
  into window j's first chunks so next-window scores never wait on
  them).  Scores are computed transposed (S^T = K Q^T) with two heads
  row-packed per matmul; each head's V tile carries 64 appended ones
  columns so the AV matmul itself writes the softmax denominator
  replicated across psum partitions 64..127 (a free partition-broadcast
  feeding a lane-parallel reciprocal+multiply normalize).  Diagonal
  chunks trim causally-dead columns from the score matmul, exp, and AV
  matmul; the final window normalizes and projects each 128-row column
  block as soon as its AV accumulation is final.
"""

import numpy as np
import ml_dtypes

BF16 = ml_dtypes.bfloat16

B, T, E = 2, 2048, 1024
H, HD = 16, 64
G = 4             # head groups (cores per batch)
HL = H // G       # heads per core
DL = HL * HD      # local qkv dim = 256
TC = 512          # T chunk (matmul moving free dim)
NJ = T // TC      # 4 q-windows
KC = 128          # k-chunk (contraction tile for attention)
NC_ = T // KC     # 16 k-chunks
SCALE = 1.0 / float(np.sqrt(HD))

_CACHE = {}


def _build_bass():
    import concourse.mybir as mybir
    import concourse.tile as tile
    from concourse import bacc

    f32 = mybir.dt.float32
    bf16 = mybir.dt.bfloat16
    EXP = mybir.ActivationFunctionType.Exp

    nc = bacc.Bacc("TRN2", target_bir_lowering=False, debug=False)
    # window-major contiguous layouts: one big DMA per tensor/window
    xtw_d = [nc.dram_tensor(f"xtw{j}", [128, 8 * TC], bf16,
                            kind="ExternalInput").ap() for j in range(NJ)]
    wqk_d = nc.dram_tensor("wqk", [128, 8 * 512], bf16, kind="ExternalInput").ap()
    wv_d = nc.dram_tensor("wv", [128, 8 * 256], bf16, kind="ExternalInput").ap()
    wo_d = nc.dram_tensor("wo", [DL, E], bf16, kind="ExternalInput").ap()
    cos_d = nc.dram_tensor("cosf", [128, T], bf16, kind="ExternalInput").ap()
    sin_d = nc.dram_tensor("sinf", [128, T], bf16, kind="ExternalInput").ap()
    rm_d = nc.dram_tensor("rmat", [128, 128], bf16, kind="ExternalInput").ap()
    y_d = nc.dram_tensor("y", [T, E], bf16, kind="ExternalOutput").ap()

    NKK = E // KC  # 8 contraction chunks for the projections

    with tile.TileContext(nc) as tc:
        with (
            tc.tile_pool(name="consts", bufs=1) as consts,
            tc.tile_pool(name="stp", bufs=2, space="PSUM") as stp,     # proj/scores/yp
            tc.tile_pool(name="avp", bufs=1, space="PSUM") as avp,     # v proj / AV
            tc.tile_pool(name="sbp", bufs=3) as sbp,                   # plain bf16
            tc.tile_pool(name="tmp_sb", bufs=3) as tmp_sb,             # rope tmps
            tc.tile_pool(name="est_sb", bufs=8) as est_sb,
            tc.tile_pool(name="attn_sb", bufs=4) as attn_sb,
            tc.tile_pool(name="ysb_p", bufs=2) as ysb_p,
            tc.tile_pool(name="small_sb", bufs=3) as small_sb,
        ):
            # ---- constant tiles (one big contiguous DMA each, ordered
            # by earliest first use; ~1MB transfers run near peak BW) ----
            wqk = consts.tile([128, 8 * 512], bf16, tag="wqk")
            wv = consts.tile([128, 8 * 256], bf16, tag="wv")
            xtw = [consts.tile([128, 8 * TC], bf16, tag=f"xtw{j}",
                               name=f"xtw{j}") for j in range(NJ)]
            cosf = consts.tile([128, T], bf16, tag="cosf")
            sinf = consts.tile([128, T], bf16, tag="sinf")
            rmat = consts.tile([128, 128], bf16, tag="rmat")

            HW = 4 * TC
            nc.sync.dma_start(out=rmat, in_=rm_d)
            nc.sync.dma_start(out=xtw[0][:, 0:HW], in_=xtw_d[0][:, 0:HW])
            nc.sync.dma_start(out=wqk[:, 0:HW], in_=wqk_d[:, 0:HW])
            nc.sync.dma_start(out=xtw[0][:, HW:2 * HW], in_=xtw_d[0][:, HW:2 * HW])
            nc.sync.dma_start(out=wqk[:, HW:2 * HW], in_=wqk_d[:, HW:2 * HW])
            nc.sync.dma_start(out=cosf[:, 0:TC], in_=cos_d[:, 0:TC])
            nc.sync.dma_start(out=sinf[:, 0:TC], in_=sin_d[:, 0:TC])
            nc.sync.dma_start(out=wv, in_=wv_d)
            nc.sync.dma_start(out=xtw[1], in_=xtw_d[1])
            nc.sync.dma_start(out=cosf[:, TC:2 * TC], in_=cos_d[:, TC:2 * TC])
            nc.sync.dma_start(out=sinf[:, TC:2 * TC], in_=sin_d[:, TC:2 * TC])
            wo = []
            for tau in range(2):
                t = consts.tile([128, E], bf16, tag=f"wo{tau}", name=f"wo{tau}")
                nc.sync.dma_start(out=t, in_=wo_d[tau * 128:(tau + 1) * 128, :])
                wo.append(t)
            nc.sync.dma_start(out=xtw[2], in_=xtw_d[2])
            nc.sync.dma_start(out=cosf[:, 2 * TC:3 * TC], in_=cos_d[:, 2 * TC:3 * TC])
            nc.sync.dma_start(out=sinf[:, 2 * TC:3 * TC], in_=sin_d[:, 2 * TC:3 * TC])
            nc.sync.dma_start(out=xtw[3], in_=xtw_d[3])
            nc.sync.dma_start(out=cosf[:, 3 * TC:], in_=cos_d[:, 3 * TC:])
            nc.sync.dma_start(out=sinf[:, 3 * TC:], in_=sin_d[:, 3 * TC:])

            def w_qk(kk, cc):
                """lhsT slice of the q/k weights: contraction rows kk*128,
                output columns cc..cc+128 of [q0 q1 k0 k1]."""
                return wqk[:, kk * 512 + cc:kk * 512 + cc + 128]

            def xt_s(c, kk):
                """[128, 128] xT block: T rows c*128.., contraction kk*128."""
                jj, tt = divmod(c, 4)
                return xtw[jj][:, kk * TC + tt * KC:kk * TC + (tt + 1) * KC]

            # triangular band mask: band[p, f] = 1 if f >= p else 0; stored
            # twice side by side so one strided mul covers both head
            # column-windows of an est tile
            band2 = consts.tile([128, 2 * KC], bf16, tag="band2")
            nc.gpsimd.memset(band2, 1.0)
            for wdup in range(2):
                nc.gpsimd.affine_select(
                    out=band2[:, wdup * KC:(wdup + 1) * KC],
                    in_=band2[:, wdup * KC:(wdup + 1) * KC],
                    compare_op=mybir.AluOpType.is_ge, fill=0.0,
                    base=0, pattern=[[1, KC]], channel_multiplier=-1,
                )

            # resident projection outputs (natural head-contiguous layout)
            qn = [[consts.tile([128, TC], bf16, tag=f"qn{tau}_{j}",
                               name=f"qn{tau}_{j}") for j in range(NJ)]
                  for tau in range(2)]
            kn = [[consts.tile([128, TC], bf16, tag=f"kn{tau}_{j}",
                               name=f"kn{tau}_{j}") for j in range(NJ)]
                  for tau in range(2)]
            # v tiles: per head 64 data columns + 64 ones columns, so the AV
            # matmul itself replicates the softmax denominator across psum
            # partitions 64..127 (free partition-broadcast)
            vsb = [consts.tile([128, HL * 128], bf16, tag=f"v{c}", name=f"v{c}")
                   for c in range(NC_)]
            for c in range(NC_):
                vv = vsb[c].rearrange("p (h d) -> p h d", h=HL)
                nc.gpsimd.memset(vv[:, :, 64:128], 1.0)

            # ---------------- projection emitters ----------------
            # The rotate-half permutation matmul of part p is deferred (it
            # depends on an ACT copy of part p's psum; the in-order PE
            # queue would stall on it if emitted immediately).
            rot_pend = []

            def flush_rot(keep=0):
                while len(rot_pend) > keep:
                    pr, sb, ta, tb, dstt, js = rot_pend.pop(0)
                    nc.tensor.matmul(pr, lhsT=rmat, rhs=sb,
                                     start=True, stop=True)
                    nc.vector.tensor_mul(ta, sb, cosf[:, js])
                    nc.vector.tensor_mul(tb, pr, sinf[:, js])
                    nc.vector.tensor_add(dstt, ta, tb)

            proj_pend = {}

            def emit_proj_qk(jj, base, tau, dst, half):
                """Half a plain projection (4 contraction chunks); the
                second half adds the bf16 copy + deferred rotate_half matmul
                + RoPE combine into dst[tau][jj]."""
                js = slice(jj * TC, (jj + 1) * TC)
                cc = base + 128 * tau
                key = (jj, base, tau)
                if half == 0:
                    p2 = stp.tile([128, 2 * TC], f32, tag="st",
                                  name=f"pp{base}_{tau}_{jj}")
                    proj_pend[key] = p2
                else:
                    p2 = proj_pend.pop(key)
                ps = p2[:, 0:TC]
                pr = p2[:, TC:2 * TC]
                for kk in range(4 * half, 4 * half + 4):
                    nc.tensor.matmul(
                        ps, lhsT=w_qk(kk, cc),
                        rhs=xtw[jj][:, kk * TC:(kk + 1) * TC],
                        start=(kk == 0), stop=(kk == NKK - 1))
                if half == 0:
                    return
                sb = sbp.tile([128, TC], bf16, tag="sb", name=f"sb{base}_{tau}_{jj}")
                nc.vector.tensor_copy(sb, ps)
                ta = tmp_sb.tile([128, TC], bf16, tag="ropeA",
                                 name=f"ra{base}_{tau}_{jj}")
                tb = tmp_sb.tile([128, TC], bf16, tag="ropeB",
                                 name=f"rb{base}_{tau}_{jj}")
                rot_pend.append((pr, sb, ta, tb, dst[tau][jj], js))
                flush_rot(keep=1)

            def emit_proj_v(jj, half):
                """v projection for T chunks [jj*4 + 2*half, +2)."""
                js0 = jj * (TC // KC) + 2 * half
                ps = stp.tile([128, 2 * TC], f32, tag="st",
                              name=f"pv{jj}_{half}")
                for tt in range(2):
                    c = js0 + tt
                    for kk in range(NKK):
                        nc.tensor.matmul(
                            ps[:, tt * DL:(tt + 1) * DL],
                            lhsT=xt_s(c, kk),
                            rhs=wv[:, kk * 256:(kk + 1) * 256],
                            start=(kk == 0), stop=(kk == NKK - 1))
                for tt in range(2):
                    c = js0 + tt
                    vv = vsb[c].rearrange("p (h d) -> p h d", h=HL)
                    src = ps[:, tt * DL:(tt + 1) * DL].rearrange(
                        "p (h d) -> p h d", h=HL)
                    nc.vector.tensor_copy(vv[:, :, 0:64], src)

            # ---------------- attention emitters ----------------
            ats = [None] * NJ
            av4s = [None] * NJ
            av_ests = [dict() for _ in range(NJ)]

            def emit_score(j, c):
                """Score matmuls + exp + diagonal band mask for chunk c."""
                d = c - 4 * j          # 0..3 on the diagonal
                coff = KC * d if d > 0 else 0
                sts = [stp.tile([128, 2 * TC], f32, tag="st",
                                name=f"st{j}_{c}_{i}") for i in range(2)]
                # one K=64 matmul per head; consecutive MMs use disjoint
                # row-groups of the PE array so they run concurrently
                for tau in range(2):
                    for ll in range(2):
                        h = 2 * tau + ll
                        stt = sts[h // 2]
                        w0_ = (h % 2) * TC
                        nc.tensor.matmul(
                            stt[:, w0_ + coff:w0_ + TC],
                            lhsT=kn[tau][c // 4][
                                64 * ll:64 * ll + 64,
                                (c % 4) * KC:(c % 4 + 1) * KC],
                            rhs=qn[tau][j][64 * ll:64 * ll + 64, coff:],
                            start=True, stop=True,
                            tile_position=(64 * ll, 0))
                ests = []
                for i in range(2):
                    est = est_sb.tile([128, 2 * TC], bf16, tag="est",
                                      name=f"est{j}_{c}_{i}")
                    if coff:
                        nc.scalar.activation(
                            est.rearrange("p (w c) -> p w c", w=2)[
                                :, :, coff:],
                            sts[i].rearrange("p (w c) -> p w c", w=2)[
                                :, :, coff:],
                            EXP, scale=SCALE)
                    else:
                        nc.scalar.activation(est, sts[i], EXP, scale=SCALE)
                    ests.append(est)
                if d >= 0:
                    b2 = band2.rearrange("p (w c) -> p w c", w=2)
                    for i in range(2):
                        ev = ests[i].rearrange("p (w c) -> p w c", w=2)[
                            :, :, KC * d:KC * (d + 1)]
                        nc.vector.tensor_mul(ev, ev, b2)
                av_ests[j][c] = ests

            def emit_av(j, c):
                """One AV accumulation step (4 heads) for window j chunk c."""
                nch = 4 * (j + 1)
                if c == 0:
                    av4s[j] = avp.tile([128, 4 * TC], f32, tag="av",
                                       name=f"av_{j}")
                av4 = av4s[j]
                coff = KC * (c - 4 * j) if c > 4 * j else 0
                for h in range(HL):
                    nc.tensor.matmul(
                        av4[0:128, h * TC + coff:(h + 1) * TC],
                        lhsT=vsb[c][:, 128 * h:128 * h + 128],
                        rhs=av_ests[j][c][h // 2][:, (h % 2) * TC + coff:
                                                  (h % 2 + 1) * TC],
                        start=(c == 0), stop=(c == nch - 1))

            def emit_y(jj, tt, cast_eng=None, warm_fill=False):
                """Output projection for q rows [jj*TC + tt*128, +128): full
                E columns, one contiguous DMA.  warm_fill runs two throwaway
                matmuls into the same psum tile first: they depend only on
                resident tiles, so they execute during the normalize-chain
                wait and keep the PE clock warm for the real matmuls (which
                overwrite them with start=True)."""
                at = ats[jj]
                yp = stp.tile([128, 2 * TC], f32, tag="st", name=f"yp{jj}_{tt}")
                if warm_fill:
                    for _ in range(2):
                        nc.tensor.matmul(
                            yp[:, 0:TC], lhsT=kn[1][3][:, 0:128],
                            rhs=qn[1][3][:, 0:TC], start=True, stop=True)
                for tau in range(2):
                    for n in range(2):
                        nc.tensor.matmul(
                            yp[:, n * TC:(n + 1) * TC],
                            lhsT=at[:, tau * TC + tt * KC:tau * TC + (tt + 1) * KC],
                            rhs=wo[tau][:, n * TC:(n + 1) * TC],
                            start=(tau == 0), stop=(tau == 1))
                ys = ysb_p.tile([128, 2 * TC], bf16, tag="y", name=f"ys{jj}_{tt}")
                if cast_eng is nc.scalar:
                    nc.scalar.copy(ys, yp)
                else:
                    nc.vector.tensor_copy(ys, yp)
                # output rides the sync HWDGE ring (input is done by the
                # time the first projection lands; keeping these off the
                # scalar queue protects the exp cadence from DMA-sem waits)
                nc.sync.dma_start(
                    out=y_d[jj * TC + tt * KC:jj * TC + (tt + 1) * KC, :],
                    in_=ys)

            def emit_norm(j, dc_scalar=False):
                """Softmax normalize window j: the AV matmul already
                replicated the denominator on psum partitions 64..127; copy
                it to a base-0 SBUF tile (reciprocal_approx requires that).
                Head pairs (0,2)/(1,3) occupy one row-half x full width of
                the merged at tile, so each pair is one copy/recip/mul."""
                at = ats[j]
                av4 = av4s[j]
                avh = av4.rearrange("p (g w) -> p g w", g=2)
                for l in range(2):     # l = row half = heads (l, l+2)
                    dc = small_sb.tile([64, 2 * TC], f32, tag=f"dc{l}",
                                       name=f"dc{j}_{l}")
                    if dc_scalar:
                        nc.scalar.copy(
                            dc.rearrange("p (g w) -> p g w", g=2),
                            avh[64:128, :, l * TC:(l + 1) * TC])
                    else:
                        nc.vector.tensor_copy(
                            dc.rearrange("p (g w) -> p g w", g=2),
                            avh[64:128, :, l * TC:(l + 1) * TC])
                    rd = small_sb.tile([64, 2 * TC], f32, tag=f"rd{l}",
                                       name=f"rd{j}_{l}")
                    nc.vector.reciprocal_approx_fast(out=rd, in_=dc)
                    nc.vector.tensor_mul(
                        at[64 * l:64 * l + 64, :].rearrange(
                            "p (g w) -> p g w", g=2),
                        avh[0:64, :, l * TC:(l + 1) * TC],
                        rd.rearrange("p (g w) -> p g w", g=2))

            def keep_warm(n=2, name="kw"):
                t = avp.tile([128, 4 * TC], f32, tag="av", name=name)
                for i in range(n):
                    nc.tensor.matmul(
                        t[:, (i % 2) * TC:(i % 2) * TC + TC],
                        lhsT=kn[1][3][:, 0:128], rhs=qn[1][3][:, 0:TC],
                        start=True, stop=True)

            # ---------------- prologue: window-0 projections ----------------
            keep_warm(16, name="kw_pro")
            for tau in range(2):
                emit_proj_qk(0, 0, tau, qn, 0)
                emit_proj_qk(0, DL, tau, kn, 0)
            for tau in range(2):
                emit_proj_qk(0, 0, tau, qn, 1)
                emit_proj_qk(0, DL, tau, kn, 1)
            emit_proj_v(0, 0)
            emit_proj_v(0, 1)
            flush_rot()

            # ---------------- pipelined window loop ----------------
            pre_scored = set()
            for j in range(NJ):
                nch = 4 * (j + 1)
                ats[j] = attn_sb.tile([128, 2 * TC], bf16, tag="attn",
                                      name=f"attn_{j}")
                flush_rot()

                side = {c: [] for c in range(nch)}
                if j > 0:
                    pn = 4 * j
                    side[0].append(lambda p=j - 1, c=pn - 2: emit_av(p, c))
                    side[0].append(lambda p=j - 1, c=pn - 1: emit_av(p, c))
                    side[1].append(lambda p=j - 1: emit_norm(p))
                    # previous window's output projections after normalize;
                    # window 0's are deferred to w3, whose scalar-bound
                    # chunks have PE slack (w1 is the most PE-dense window)
                    if j != 1:
                        ystep = max(1, (nch - 3) // 4)
                        for tt in range(4):
                            side[min(nch - 1, 3 + tt * ystep)].append(
                                lambda p=j - 1, t=tt: emit_y(p, t))
                    if j == NJ - 1:
                        for tt in range(4):
                            side[4 + 3 * tt].append(
                                lambda t=tt: emit_y(0, t))
                    # own v projection (vsb[4j..4j+3] first used by the AV
                    # at chunk 4j+2): scheduled as late as its deadline
                    # allows to smooth the PE load across the window
                    side[max(0, 4 * j - 1)].append(
                        lambda p=j: emit_proj_v(p, 0))
                    side[4 * j].append(
                        lambda p=j: emit_proj_v(p, 1))
                if j + 1 < NJ:
                    # q/k projections for the next window
                    porder = [(0, 0), (DL, 0), (0, 1), (DL, 1)]
                    pstep = max(1, (nch - 2) // 4)
                    for i, (base, tau) in enumerate(porder):
                        dst = qn if base == 0 else kn
                        side[min(nch - 2, i * pstep)].append(
                            lambda b=base, t=tau, d=dst, p=j + 1:
                            emit_proj_qk(p, b, t, d, 0))
                        side[min(nch - 2, i * pstep)].append(
                            lambda b=base, t=tau, d=dst, p=j + 1:
                            emit_proj_qk(p, b, t, d, 1))
                    # prefetch the next window's first score chunk at this
                    # window's last chunk: its exp bridges the boundary gap
                    # on the scalar engine (placed only after window j's own
                    # last score so the psum rotation order is preserved)
                    def pre_score(p=j + 1):
                        flush_rot()     # qn/kn[p] combines must be emitted
                        emit_score(p, 0)
                        emit_score(p, 1)
                        pre_scored.add((p, 0))
                        pre_scored.add((p, 1))
                    side[nch - 1].append(pre_score)

                for c in range(nch):
                    if (j, c) not in pre_scored:
                        emit_score(j, c)
                    for t in side[c]:
                        t()
                    # flush any pending rotate-half matmul now: deferring it
                    # to the next part would hold its psum tile across
                    # chunks, and the 2-buffer rotation would make upcoming
                    # score matmuls wait on that release
                    flush_rot()
                    if c >= 2:
                        emit_av(j, c - 2)

            # ---------------- tail: window 3 ----------------
            # column block tt of the AV psum is final once av(j, 12+tt) has
            # run (later diagonal chunks only touch columns beyond it), so
            # each 128-q block normalizes + projects while later chunks are
            # still accumulating -- the PE never idles long enough to lose
            # its HAM warm state
            j = NJ - 1
            nch = 4 * NJ

            def tail_tt(tt):
                at = ats[j]
                av4 = av4s[j]
                avh = av4.rearrange("p (g w) -> p g w", g=2)
                dc = small_sb.tile([64, HL * KC], f32, tag="dct",
                                   name=f"dct{tt}")
                nc.scalar.copy(
                    dc.rearrange("p (h c) -> p h c", h=HL),
                    av4.rearrange("p (h c) -> p h c", h=HL)[
                        64:128, :, tt * KC:(tt + 1) * KC])
                rd = small_sb.tile([64, HL * KC], f32, tag="rdt",
                                   name=f"rdt{tt}")
                nc.vector.reciprocal_approx_fast(out=rd, in_=dc)
                rdh = rd.rearrange("p (g l c) -> p l g c", g=2, l=2)
                ath = at.rearrange("p (g w) -> p g w", g=2)
                for l in range(2):     # heads (l, l+2)
                    nc.vector.tensor_mul(
                        ath[64 * l:64 * l + 64, :,
                            tt * KC:(tt + 1) * KC],
                        avh[0:64, :,
                            l * TC + tt * KC:l * TC + (tt + 1) * KC],
                        rdh[:, l, :, :])
                emit_y(j, tt, cast_eng=nc.scalar)

            emit_av(j, nch - 2)
            tail_tt(0)
            emit_av(j, nch - 1)
            tail_tt(1)
            tail_tt(2)
            tail_tt(3)

    nc.compile()
    return nc


def _host_inputs(x, cos, sin, w_qkv, w_out):
    """Shard + lay out the full inputs for the 8 cores."""
    # natural-layout tables: row 64*l + d = cos/sin[t, d]
    cosf = np.ascontiguousarray(np.tile(cos.T, (2, 1))).astype(BF16)
    sinf = np.ascontiguousarray(np.tile(sin.T, (2, 1))).astype(BF16)

    # xtw[b][jj] = [128, 8*TC]: kk-major blocks of the T-window jj of x^T
    xts = [x[b].T.astype(BF16) for b in range(B)]
    xtws = []
    for b in range(B):
        xt = xts[b].reshape(8, KC, NJ, TC)           # (kk, p, jj, t)
        xtws.append([np.ascontiguousarray(
            xt[:, :, jj, :].transpose(1, 0, 2).reshape(KC, 8 * TC))
            for jj in range(NJ)])

    # lhsT for the rotate_half permutation matmul: rot = rmat.T @ q with
    # rot[d] = -q[d+32] (d<32), q[d-32] (d>=32) per 64-row head block
    r64 = np.zeros((64, 64), dtype=np.float32)
    r64[np.arange(32) + 32, np.arange(32)] = -1.0
    r64[np.arange(32), np.arange(32) + 32] = 1.0
    rmat = np.zeros((128, 128), dtype=np.float32)
    rmat[0:64, 0:64] = r64
    rmat[64:128, 64:128] = r64
    rmat = rmat.astype(BF16)

    in_maps = []
    for core in range(8):
        b, g = divmod(core, G)
        qblk = w_qkv[:, G * g * HD:(G * g + HL) * HD]
        kblk = w_qkv[:, E + G * g * HD:E + (G * g + HL) * HD]
        vblk = w_qkv[:, 2 * E + DL * g:2 * E + DL * (g + 1)]
        # wqk: kk-major blocks of [q0 q1 k0 k1] (512 cols each)
        qk = np.concatenate([qblk, kblk], axis=1).astype(BF16)  # (E, 512)
        wqkl = np.ascontiguousarray(
            qk.reshape(8, KC, 512).transpose(1, 0, 2).reshape(KC, 8 * 512))
        # wv: kk-major blocks of the 256 v columns
        wvl = np.ascontiguousarray(
            vblk.astype(BF16).reshape(8, KC, 256).transpose(1, 0, 2)
            .reshape(KC, 8 * 256))
        wol = np.ascontiguousarray(w_out[DL * g:DL * (g + 1), :]).astype(BF16)
        m = {"wqk": wqkl, "wv": wvl, "wo": wol, "cosf": cosf, "sinf": sinf,
             "rmat": rmat}
        for jj in range(NJ):
            m[f"xtw{jj}"] = xtws[b][jj]
        in_maps.append(m)
    return in_maps


def kernel(x, cos, sin, w_qkv, w_out):
    from concourse import bass_utils

    if "nc" not in _CACHE:
        _CACHE["nc"] = _build_bass()
    nc = _CACHE["nc"]

    in_maps = _host_inputs(
        np.asarray(x, dtype=np.float32), np.asarray(cos, dtype=np.float32),
        np.asarray(sin, dtype=np.float32), np.asarray(w_qkv, dtype=np.float32),
        np.asarray(w_out, dtype=np.float32))

    res = bass_utils.run_bass_kernel_spmd(nc, in_maps, core_ids=list(range(8)))

    y = np.zeros((B, T, E), dtype=np.float32)
    for core in range(8):
        b = core // G
        y[b] += res.results[core]["y"].astype(np.float32)
    return y
